# revision 13
# baseline (speedup 1.0000x reference)
"""GAT attention kernel for 8 trn2 NeuronCores (Bass/Tile), bf16 edition v5.

Math (restructured from the reference to avoid materializing h_j):
    wa1 = W @ a1, wa2 = W @ a2                      (host, once, O(F^2))
    s[n,k]  = x[n,k]·wa2 - 500*(1-adj[n,k])         (mask folded into score)
    s2      = s + si,  si = x0[n]·wa1
    e       = exp(leaky_relu(s2, 0.2))              (masked: exp(~-100) ~ 0,
                                                     same as reference -9e15 mask)
    att     = (e + EPS) / (sum_k e + 16*EPS)        (== uniform 1/16 when row fully
                                                     masked, matching reference)
    xbar[n] = sum_k att[n,k] * x[n,k,:]
    out     = elu((xbar + x0) @ W)                  (since h_prime + h = (xbar + x0)@W)
    elu(z)  = min(exp(z) - 1, relu(z))

Sharding: node dim N padded 50000 -> 50176 = 8 cores * 49 tiles * 128 rows.
Per 128-row tile the 2048 (n,k) pairs form 16 blocks of [128 nk-rows, 129]
(128 features + the mask bias element) in bf16; block 17 is [x0 | 0] so the
x0·wa1 dot rides along.  Host pre-permutes + casts so the DMA is a single
contiguous ~561KB transfer per tile.

DVE has a ~151-cycle fixed cost per instruction and its segmented reduce runs
at 1x (measured (N+151)/0.96), so scores are two big DVE ops: one 2x bf16
tensor_tensor product against a precomposed [128, 17*129] weights tile
(16x [wa2|1] then [wa1|1]), then one segmented tensor_reduce(axis=X)
[128,17,129] -> [128,17] f32 (scores are pre-masked, si is col 16).

Per tile:
  DVE : prod TT, segmented reduce, att = (e+eps)*RZ, attseg = SEGBIG*att,
        recip_fast, y = min(E-1, R)
  PE  : si scatter (Cm fp32), Z group-sum (SEG), RZrep (E8), x0^T identity
        matmul + 16 bf16 xbar matmuls (accumulate xbarT in PSUM), final @W
  ACT : si_s copy, Prelu(0.2), Exp, tz copy (+16eps), ST copy, R=relu(z),
        E=exp(z)
  GPS : Dt (si broadcast mask), s2 = s+si_s
"""

import numpy as np

N, K, F = 50000, 16, 128
ALPHA = 0.2
NCORES = 8
TILE = 128
NTILES = 49
RPC = TILE * NTILES          # rows per core = 6272
BPT = K                      # nk-blocks per tile = 16
SEGL = F + 1                 # segment length: 128 features + mask bias elem
XCOLS = (BPT + 1) * SEGL     # 17 segments of 129 = 2193
WREP = XCOLS
EPS = 1e-12
MASKB = -500.0

_NC_CACHE = {}


def _consts_f32_np():
    p = np.arange(128)
    j8 = np.arange(8)
    b16 = np.arange(16)
    # C[n, q] = 1 iff n%8 == q//16   (si scatter: out[q,b] = si[8b + q//16])
    Cm = (p[:, None] % 8 == p[None, :] // 16).astype(np.float32)
    # SEG[q, j] = 1 iff q//16 == j   [128, 8]
    seg = (p[:, None] // 16 == j8[None, :]).astype(np.float32)
    # E8 rows 0..8: E8[j, q] = 1 iff q//16 == j (used as lhsT [8,128])
    e8 = ((p[:, None] < 8) & (p[None, :] // 16 == p[:, None])).astype(np.float32)
    # SEG8[n, b] = 1 iff n//8 == b   [128, 16]
    seg8 = (p[:, None] // 8 == b16[None, :]).astype(np.float32)
    return np.ascontiguousarray(
        np.concatenate([Cm, seg, e8, seg8], axis=1))  # [128, 280]


def _consts_bf16_np(W, a):
    import ml_dtypes
    bf16 = ml_dtypes.bfloat16
    p = np.arange(128)
    W = np.asarray(W, np.float32)
    a = np.asarray(a, np.float32)
    wa1 = W @ a[:F, 0]
    wa2 = W @ a[F:, 0]
    # warep [128, 17*129]: 16x [wa2 | 1.0] then [wa1 | 1.0]
    seg2 = np.concatenate([wa2, [1.0]]).astype(np.float32)
    seg1 = np.concatenate([wa1, [1.0]]).astype(np.float32)
    row = np.concatenate([np.tile(seg2, BPT), seg1])
    warep = np.broadcast_to(row, (128, XCOLS))
    ident = np.eye(128, dtype=np.float32)
    segbig = (p[:, None] // 16 == (p[None, :] % 8)).astype(np.float32)
    return np.ascontiguousarray(np.concatenate(
        [warep, W, ident, segbig], axis=1).astype(bf16))  # [128, 2193+384]


def _build_nc(ntiles=NTILES, finalize=True):
    import concourse.mybir as mybir
    import concourse.tile as tile
    from concourse import bacc

    fp = mybir.dt.float32
    bf = mybir.dt.bfloat16
    AF = mybir.ActivationFunctionType
    OP = mybir.AluOpType

    nc = bacc.Bacc("TRN2")
    xd = nc.dram_tensor("xd", [ntiles, 128, XCOLS], bf, kind="ExternalInput")
    cstf = nc.dram_tensor("cstf", [128, 280], fp, kind="ExternalInput")
    cstb = nc.dram_tensor("cstb", [128, XCOLS + 384], bf, kind="ExternalInput")
    yd = nc.dram_tensor("yd", [ntiles, 128, F], bf, kind="ExternalOutput")

    with tile.TileContext(nc) as tc:
        with (
            tc.tile_pool(name="const", bufs=1) as constp,
            tc.tile_pool(name="xin", bufs=9) as xin,
            tc.tile_pool(name="small", bufs=4) as small,
            tc.tile_pool(name="big", bufs=4) as big,
            tc.tile_pool(name="scrp", bufs=2) as scrp,
            tc.tile_pool(name="yout", bufs=3) as yout,
            tc.tile_pool(name="ps", bufs=1, space="PSUM") as ps,
        ):
            # ---------------- setup: two DMAs, no device compute ----------
            constsf = constp.tile([128, 280], fp)
            nc.sync.dma_start(out=constsf, in_=cstf[:, :])
            Cm = constsf[:, 0:128]
            SEG = constsf[:, 128:136]
            E8 = constsf[:, 136:264]
            SEG8 = constsf[:, 264:280]

            constsb = constp.tile([128, XCOLS + 384], bf, tag="cb")
            nc.sync.dma_start(out=constsb, in_=cstb[:, :])
            warep_cat = constsb[:, 0:XCOLS]
            W_bf = constsb[:, XCOLS:XCOLS + 128]
            IDENT_bf = constsb[:, XCOLS + 128:XCOLS + 256]
            SEGBIG_bf = constsb[:, XCOLS + 256:XCOLS + 384]

            # ---------------- software-pipelined tile loop ----------------
            # load(t) | score(t-2) | mask(t-3) | recip(t-4) | att/xbar(t-5)
            # | final/elu(t-6) | store(t-7)
            st = {}

            def phase_load(t):
                xall = xin.tile([128, XCOLS], bf, tag="x")
                nc.sync.dma_start(out=xall, in_=xd[t])
                st[t] = {"xall": xall}

            def phase_score(t):
                d = st[t]
                xall = d["xall"]
                # one big product + one segmented reduce: s17[:, 0:16] are
                # the 16 pre-masked neighbor scores, s17[:, 16] is si
                scr = scrp.tile([128, XCOLS], bf, tag="scr")
                nc.vector.tensor_mul(out=scr, in0=xall, in1=warep_cat)
                s17 = small.tile([128, BPT + 1], fp, tag="s17")
                nc.vector.tensor_reduce(
                    out=s17,
                    in_=scr.rearrange("p (b f) -> p b f", f=SEGL),
                    axis=mybir.AxisListType.X, op=OP.add,
                )
                Dt = small.tile([128, K], fp, tag="D")
                si_bc = s17[:, BPT:BPT + 1].rearrange(
                    "p (b o) -> p b o", o=1).to_broadcast([128, K, 1])
                nc.gpsimd.tensor_mul(
                    out=Dt.rearrange("p (b o) -> p b o", o=1),
                    in0=SEG8.rearrange("p (b o) -> p b o", o=1), in1=si_bc)
                si_ps = ps.tile([128, K], fp, tag="si", bufs=1)
                nc.tensor.matmul(si_ps, lhsT=Cm, rhs=Dt, start=True, stop=True)
                si_s = small.tile([128, K], fp, tag="si_s")
                nc.scalar.activation(out=si_s, in_=si_ps, func=AF.Copy)
                s2 = small.tile([128, K], fp, tag="s2")
                nc.gpsimd.tensor_add(out=s2, in0=s17[:, 0:BPT], in1=si_s)
                d["s2"] = s2

            def phase_mask(t):
                d = st[t]
                ls = small.tile([128, K], fp, tag="ls")
                nc.scalar.activation(out=ls, in_=d["s2"], func=AF.Prelu,
                                     alpha=ALPHA)
                exp_s = small.tile([128, K], fp, tag="exp_s")
                nc.scalar.activation(out=exp_s, in_=ls, func=AF.Exp)
                Z_ps = ps.tile([8, K], fp, tag="Z", bufs=2)
                nc.tensor.matmul(Z_ps, lhsT=SEG, rhs=exp_s, start=True, stop=True)
                tz = small.tile([8, K], fp, tag="tz")
                nc.scalar.activation(out=tz, in_=Z_ps, func=AF.Copy, bias=16.0 * EPS)
                d["p_s"] = exp_s
                d["tz"] = tz

            def phase_recip(t):
                d = st[t]
                RZ = small.tile([8, K], fp, tag="RZ")
                nc.vector.reciprocal_approx_fast(RZ, d["tz"])
                RZrep_ps = ps.tile([128, K], fp, tag="RZrep", bufs=2)
                nc.tensor.matmul(RZrep_ps, lhsT=E8[0:8, :], rhs=RZ,
                                 start=True, stop=True)
                d["RZrep"] = RZrep_ps

            def phase_xbar(t):
                d = st[t]
                xall = d["xall"]
                att = small.tile([128, K], bf, tag="att")
                nc.vector.scalar_tensor_tensor(
                    out=att, in0=d["p_s"], scalar=EPS, in1=d["RZrep"],
                    op0=OP.add, op1=OP.mult,
                )
                attseg = big.tile([128, 128], bf, tag="attseg")
                att_bc = att.rearrange("p (b o) -> p b o", o=1).to_broadcast([128, K, 8])
                nc.vector.tensor_mul(
                    out=attseg.rearrange("p (b j) -> p b j", j=8),
                    in0=SEGBIG_bf.rearrange("p (b j) -> p b j", j=8),
                    in1=att_bc,
                )
                x0_nat = xall[:, BPT * SEGL:BPT * SEGL + F]
                xbarT_ps = ps.tile([128, 128], fp, tag="mm", bufs=3)
                nc.tensor.matmul(xbarT_ps, lhsT=x0_nat,
                                 rhs=IDENT_bf, start=True, stop=False,
                                 skip_group_check=True)
                for b in range(BPT):
                    nc.tensor.matmul(
                        xbarT_ps[:, 8 * b:8 * b + 8],
                        lhsT=xall[:, b * SEGL:b * SEGL + F],
                        rhs=attseg[:, 8 * b:8 * b + 8],
                        start=False, stop=(b == BPT - 1),
                        skip_group_check=True,
                    )
                d["xbarT"] = xbarT_ps

            def phase_out(t):
                d = st[t]
                ST_sb = big.tile([128, 128], bf, tag="ST")
                nc.scalar.activation(out=ST_sb, in_=d["xbarT"], func=AF.Copy)
                zfin_ps = ps.tile([128, 128], fp, tag="mm", bufs=3)
                nc.tensor.matmul(zfin_ps, lhsT=ST_sb, rhs=W_bf, start=True, stop=True)
                r_sb = big.tile([128, 128], bf, tag="r")
                nc.scalar.activation(out=r_sb, in_=zfin_ps, func=AF.Relu)
                e_sb = big.tile([128, 128], bf, tag="e")
                nc.scalar.activation(out=e_sb, in_=zfin_ps, func=AF.Exp)
                d["r"] = r_sb
                d["e"] = e_sb

            def phase_store(t):
                d = st[t]
                y_sb = yout.tile([128, 128], bf, tag="y")
                nc.vector.scalar_tensor_tensor(
                    out=y_sb, in0=d["e"], scalar=-1.0, in1=d["r"],
                    op0=OP.add, op1=OP.min,
                )
                nc.sync.dma_start(out=yd[t], in_=y_sb)
                del st[t]

            for r in range(ntiles + 7):
                if r < ntiles:
                    phase_load(r)
                if 0 <= r - 2 < ntiles:
                    phase_score(r - 2)
                if 0 <= r - 3 < ntiles:
                    phase_mask(r - 3)
                if 0 <= r - 4 < ntiles:
                    phase_recip(r - 4)
                if 0 <= r - 5 < ntiles:
                    phase_xbar(r - 5)
                if 0 <= r - 6 < ntiles:
                    phase_out(r - 6)
                if 0 <= r - 7 < ntiles:
                    phase_store(r - 7)

    if finalize:
        nc.finalize()
    return nc


def _get_nc(ntiles=NTILES):
    if ntiles not in _NC_CACHE:
        _NC_CACHE[ntiles] = _build_nc(ntiles)
    return _NC_CACHE[ntiles]


def _shard_inputs(orignal_x, x, adj, W, a, ncores=NCORES, ntiles=NTILES):
    import ml_dtypes
    bf16 = ml_dtypes.bfloat16
    f32 = np.float32
    rpc = TILE * ntiles
    n_used = rpc * ncores
    x = np.asarray(x, f32)
    x0 = np.asarray(orignal_x, f32)
    adj = np.asarray(adj, np.int32)
    cf = _consts_f32_np()
    cb = _consts_bf16_np(W, a)
    n = x.shape[0]

    in_maps = []
    for c in range(ncores):
        lo = c * rpc
        hi = min((c + 1) * rpc, n)
        rows = hi - lo
        xc = x[lo:hi]
        x0c = x0[lo:hi]
        adjc = adj[lo:hi]
        if rows < rpc:
            pad = rpc - rows
            xc = np.concatenate([xc, np.zeros((pad, K, F), f32)])
            x0c = np.concatenate([x0c, np.zeros((pad, F), f32)])
            adjc = np.concatenate([adjc, np.zeros((pad, K), np.int32)])
        # per-tile layout [t, q, b*129 + f] (s-layout blocks of 129: 128
        # features + mask bias elem); block 17 = [x0 | 0]
        xdev = np.empty((ntiles, 128, XCOLS), bf16)
        xs = xdev.reshape(ntiles, 128, BPT + 1, SEGL)
        xs[:, :, :BPT, :F] = xc.reshape(ntiles, BPT, 128, F).transpose(
            0, 2, 1, 3).astype(bf16)
        # mask bias element: -500 where adj==0, else 0 (s-layout)
        mb = (MASKB * (1 - adjc)).astype(f32).reshape(
            ntiles, BPT, 128).transpose(0, 2, 1)
        xs[:, :, :BPT, F] = mb.astype(bf16)
        xs[:, :, BPT, :F] = x0c.reshape(ntiles, 128, F).astype(bf16)
        xs[:, :, BPT, F] = bf16(0.0)
        in_maps.append({
            "xd": xdev,
            "cstf": cf,
            "cstb": cb,
        })
    assert n <= n_used
    return in_maps


_LAST_RESULTS = None


def kernel(orignal_x, x, adj, W, a):
    import os
    os.environ.setdefault("JAX_PLATFORMS", "")
    from concourse.bass_utils import run_bass_kernel_spmd

    global _LAST_RESULTS
    nc = _get_nc()
    in_maps = _shard_inputs(orignal_x, x, adj, W, a)
    res = run_bass_kernel_spmd(nc, in_maps, list(range(NCORES)))
    _LAST_RESULTS = res
    y = np.concatenate(
        [np.asarray(r["yd"], np.float32).reshape(RPC, F) for r in res.results],
        axis=0)
    return np.ascontiguousarray(y[:N])
